# revision 39
# baseline (speedup 1.0000x reference)
"""Multi-head attention (B=2, S=2048, D=1024, H=16, RoPE, causal) on 8 trn2 cores.

Sharding: core = b*4 + g  ->  batch b in {0,1}, head-group g (4 heads of 64 dims).
Each core computes q/k/v projections for its 4 heads, RoPE, causal attention,
and a partial output projection (its 256 rows of wo). Host sums the 4 partials
per batch and adds the analytic bias correction bv@wo + bo (softmax rows sum
to 1, so bv contributes a constant vector; bo is a constant vector).

Device layouts are feature-on-partition ("transposed"):
  xt [128, 8, 2048]   xt[p, ko, s] = x[b, s, ko*128 + p]            (bf16)
  qT/kT computed directly as [d', s]; RoPE pair-swap becomes a 32-block
  partition swap because wq/wk columns are host-permuted to [evens|odds]
  per head. The swap runs as 4 SBUF->SBUF partition-block DMAs (p ^ 32),
  freeing the PE of permutation matmuls. The softmax 1/sqrt(64) scale is
  folded into wq/bq on the host so q and k share one cos/sin table pair
  (duplicated along a middle dim so one DVE op covers q and k together).
  scoresT[j, i] = kT.T @ qT per head; two heads (one 128-part chunk) run as
  concurrent K=64 row-group matmuls into the two banks of one [128,1024]
  PSUM tile, so exp / mask / normalize handle both heads per op.
  Softmax skips max-subtraction (|score| <~ 8 here); exp on ACT. The
  denominator comes free from a ones-column appended to v in the AV matmul
  (out rows 0..63 = v.T @ attnT, row 64 = column sums). The reciprocal runs
  on the DVE (reciprocal_approx_fast) so the ACT queue carries nothing but
  the block exps; the normalize multiplies run on the otherwise-idle GpSimd.
  Causality: blocks above the diagonal are skipped, diagonal blocks compute
  only columns >= 128*r and mask a single 128-wide strip via a -30 matmul.
  y[s, e] = outT.T @ wo accumulated over the 2 c-chunks, DMA'd PSUM->DRAM.
"""

import os

import numpy as np
import ml_dtypes

import concourse.bass as bass
import concourse.bacc as bacc
import concourse.tile as tile
from concourse import library_config, mybir

B = 2
S = 2048
D = 1024
H = 16
HD = 64
NCORES = 8
HEADS_PER_CORE = 4
DP = 256  # head dims per core
SEG = 512  # i-seg / s-seg size
NSEG = S // SEG  # 4
NST = S // 128  # 16 s-tiles / j-tiles
KO = D // 128  # 8 contraction tiles

F32 = mybir.dt.float32
BF16 = mybir.dt.bfloat16

_PROGRAMS = {}


def _build_program(with_qk_bias):
    nc = bacc.Bacc("TRN2", target_bir_lowering=False, debug=False)

    xt_d = nc.dram_tensor("xt", [128, KO, S], BF16, kind="ExternalInput")
    wq_d = nc.dram_tensor("wqt", [128, KO, DP], BF16, kind="ExternalInput")
    wk_d = nc.dram_tensor("wkt", [128, KO, DP], BF16, kind="ExternalInput")
    wv_d = nc.dram_tensor("wvt", [128, KO, DP], BF16, kind="ExternalInput")
    wo_d = nc.dram_tensor("wot", [128, 2, D], BF16, kind="ExternalInput")
    bq_d = nc.dram_tensor("bqt", [128, 2], F32, kind="ExternalInput")
    bk_d = nc.dram_tensor("bkt", [128, 2], F32, kind="ExternalInput")
    cos_d = nc.dram_tensor("cos2", [128, 2, S], BF16, kind="ExternalInput")
    sin_d = nc.dram_tensor("sin2", [128, 2, S], BF16, kind="ExternalInput")
    cm_d = nc.dram_tensor("cmask", [128, 128], BF16, kind="ExternalInput")
    i128_d = nc.dram_tensor("i128", [128, 128], BF16, kind="ExternalInput")
    y_d = nc.dram_tensor("y", [S, D], BF16, kind="ExternalOutput")
    # co=0 partial of the last segment's y, emitted during the last chunk's
    # attention so the post-norm tail only runs the co=1 half (host adds it)
    y2_d = nc.dram_tensor("y2", [SEG, D], BF16, kind="ExternalOutput")

    with tile.TileContext(nc) as tc:
        with (
            tc.tile_pool(name="const", bufs=1) as const,
            tc.tile_pool(name="persist", bufs=1) as persist,
            tc.tile_pool(name="work", bufs=4) as work,
            tc.tile_pool(name="psmm", bufs=2, space="PSUM") as psmm,
            tc.tile_pool(name="pssc", bufs=2, space="PSUM") as pssc,
            tc.tile_pool(name="psacc", bufs=1, space="PSUM") as psacc,
        ):
            # one gpsimd library serving BOTH partition_broadcast and
            # tensor_tensor: a mid-kernel LIBRARY_RELOAD costs ~6us of
            # hidden ucode DMA, so the normalize chain must never thrash
            nc.gpsimd.load_library(library_config.proxy)

            # ---- constants ----  (DMA order = first-needed first)
            wq = const.tile([128, KO, DP], BF16, tag="wq")
            nc.sync.dma_start(wq[:], wq_d[:])
            xt = []
            xt0h = []  # first seg split in two ko-halves for an early start
            for h in range(2):
                xh = const.tile([128, KO // 2, SEG], BF16, tag=f"xt0h{h}",
                                name=f"xt0h{h}")
                nc.sync.dma_start(
                    xh[:], xt_d[:, h * (KO // 2):(h + 1) * (KO // 2), 0:SEG])
                xt0h.append(xh)
            for t in range(NSEG):
                if t == 0:
                    xt.append(None)
                    continue
                xt.append(const.tile([128, KO, SEG], BF16, tag=f"xt{t}",
                                     name=f"xt{t}"))
            wk = const.tile([128, KO, DP], BF16, tag="wk")
            nc.sync.dma_start(wk[:], wk_d[:])

            def xt_ap(t, ko):
                if t == 0:
                    return xt0h[ko // (KO // 2)][:, ko % (KO // 2), :]
                return xt[t][:, ko, :]
            if with_qk_bias:
                bq = const.tile([128, 2], F32, tag="bq")
                nc.sync.dma_start(bq[:], bq_d[:])
                bk = const.tile([128, 2], F32, tag="bk")
                nc.sync.dma_start(bk[:], bk_d[:])
            ctab = const.tile([128, 2, S], BF16, tag="ctab")
            nc.sync.dma_start(ctab[:], cos_d[:])
            stab = const.tile([128, 2, S], BF16, tag="stab")
            nc.sync.dma_start(stab[:], sin_d[:])
            wv = const.tile([128, KO, DP], BF16, tag="wv")
            nc.sync.dma_start(wv[:], wv_d[:])
            cm = const.tile([128, 128], BF16, tag="cm")
            nc.sync.dma_start(cm[:], cm_d[:])
            i128 = const.tile([128, 128], BF16, tag="i128")
            nc.sync.dma_start(i128[:], i128_d[:])
            for t in range(1, NSEG):
                nc.sync.dma_start(xt[t][:], xt_d[:, :, t * SEG:(t + 1) * SEG])
            wo = const.tile([128, 2, D], BF16, tag="wo")
            nc.sync.dma_start(wo[:], wo_d[:])

            # ---- PE warmup: ~4.5us of dummy matmuls while DMAs stream,
            # so the HAM clock-gate is at 8/8 when real work starts ----
            wmt = work.tile([128, 128], BF16, tag="wmt")
            nc.vector.memset(wmt[:], 0.0)
            wps = psmm.tile([128, SEG], F32, tag="mm", name="warm")
            for w in range(48):
                nc.tensor.matmul(wps[:, 0:128], wmt[:], wmt[:],
                                 start=(w == 0), stop=(w == 47))

            # ---- per-segment pipeline: projections -> attention -> y ----
            rot = {}        # (c, t) -> [128, 2(q/k), SEG] bf16
            v2 = [None] * (NST // 2)  # half-seg -> [128, 2, 4, 66] bf16
            outt = {}

            def _emit_y(yt, cos=(0, 1), ydst=None, yrow0=None):
                if yt < 0:
                    return
                for sl in range(4):
                    st = 4 * yt + sl
                    for es in range(2):
                        py = psmm.tile([128, SEG], F32, tag="mm",
                                       name=f"py_{st}_{es}_{cos[0]}")
                        for i, co in enumerate(cos):
                            nc.tensor.matmul(
                                py[:],
                                outt[(co, yt)][:, sl * 128:sl * 128 + 128],
                                wo[:, co, es * SEG:(es + 1) * SEG],
                                start=(i == 0), stop=(i == len(cos) - 1))
                        ysb = work.tile([128, SEG], BF16, tag="ysb")
                        nc.any.tensor_copy(ysb[:], py[:])
                        dst = y_d if ydst is None else ydst
                        r0 = st * 128 if yrow0 is None else yrow0 + sl * 128
                        nc.sync.dma_start(
                            dst[r0:r0 + 128, es * SEG:(es + 1) * SEG],
                            ysb[:])

            def proj_steps(t):
                """Generator: q/k/v projections + rope for segment t,
                yielded in small PE-group steps so the caller can weave
                them between attention blocks (filler for the exp-gated
                AV waits)."""
                for c in range(2):
                    qk_sb = work.tile([128, 2, SEG], BF16, tag="qk_sb")
                    for qk, (w_sb, b_nm) in enumerate(
                            ((wq, "bq"), (wk, "bk"))):
                        pp = psmm.tile([128, SEG], F32, tag="mm",
                                       name=f"pp{qk}_{c}_{t}")
                        for ko in range(KO):
                            nc.tensor.matmul(
                                pp[:],
                                w_sb[:, ko, c * 128:(c + 1) * 128],
                                xt_ap(t, ko),
                                start=(ko == 0),
                                stop=(ko == KO - 1),
                            )
                            if ko % 3 == 2:
                                yield
                        if with_qk_bias:
                            b_sb = bq if b_nm == "bq" else bk
                            nc.vector.tensor_scalar_add(
                                qk_sb[:, qk, :], pp[:], b_sb[:, c:c + 1])
                        else:
                            nc.any.tensor_copy(qk_sb[:, qk, :], pp[:])
                        yield
                    # pair-swap (p ^ 32) via 4 SBUF->SBUF partition DMAs
                    swp = work.tile([128, 2, SEG], BF16, tag="swp")
                    for g in range(4):
                        src = g ^ 1
                        nc.sync.dma_start(
                            swp[g * 32:(g + 1) * 32, :, :],
                            qk_sb[src * 32:(src + 1) * 32, :, :])
                    t1 = work.tile([128, 2, SEG], BF16, tag="t1")
                    nc.vector.tensor_tensor(
                        t1[:], qk_sb[:], ctab[:, :, t * SEG:(t + 1) * SEG],
                        mybir.AluOpType.mult)
                    t2 = work.tile([128, 2, SEG], BF16, tag="t2")
                    nc.vector.tensor_tensor(
                        t2[:], swp[:], stab[:, :, t * SEG:(t + 1) * SEG],
                        mybir.AluOpType.mult)
                    rt = persist.tile([128, 2, SEG], BF16, tag=f"rot_{c}_{t}")
                    nc.vector.tensor_tensor(
                        rt[:], t1[:], t2[:], mybir.AluOpType.add)
                    rot[(c, t)] = rt
                    yield
                # v projection for the 4 s-tiles of this seg, 2 per bank
                for half in range(2):
                    hs = 2 * t + half
                    pv = psmm.tile([128, SEG], F32, tag="mm", name=f"pv_{hs}")
                    for sl in range(2):
                        st = 4 * t + 2 * half + sl
                        for ko in range(KO):
                            nc.tensor.matmul(
                                pv[:, sl * DP:sl * DP + DP],
                                xt_ap(t, ko)[:, (st % NSEG) * 128:
                                             (st % NSEG) * 128 + 128],
                                wv[:, ko, :],
                                start=(ko == 0),
                                stop=(ko == KO - 1),
                            )
                            if ko % 3 == 2:
                                yield
                    v_t = persist.tile([128, 2, HEADS_PER_CORE, 66], BF16,
                                       tag=f"v2_{hs}")
                    nc.vector.memset(v_t[:, :, :, 64:66], 1.0)
                    nc.any.tensor_copy(
                        v_t[:, :, :, 0:64],
                        pv[:].rearrange("p (s h d) -> p s h d",
                                        s=2, h=HEADS_PER_CORE))
                    v2[hs] = v_t
                    yield

            for t in range(NSEG):
                if t == 0:
                    for _ in proj_steps(0):
                        pass
                # next segment's projections are woven into this segment's
                # attention as PE filler (consumed between scores and AV)
                filler = proj_steps(t + 1) if t + 1 < NSEG else iter(())
                nsteps = 30  # upper bound on proj_steps yields
                nblocks = 2 * (4 * t + 4)
                per_block = max(1, -(-nsteps // nblocks))
                # attention for this seg (both chunks); y(t-1) emitted
                # between the chunks to fill PE bubbles at the c=0 tail
                for c in range(2):
                    if c == 1:
                        _emit_y(t - 1)
                        if t == NSEG - 1:
                            # co=0 half of the last segment's y runs as
                            # filler during this chunk's attention
                            _emit_y(t, cos=(0,), ydst=y2_d, yrow0=0)
                    pav = psacc.tile([128, 2, SEG], F32, tag="av",
                                     name=f"av_{c}_{t}")
                    njt = 4 * t + 4
                    for jj in range(njt):
                        r = jj - 4 * t  # >= 0 on diagonal blocks
                        col0 = max(0, r) * 128  # first useful i-column
                        a = work.tile([128, 2, SEG], BF16, tag="attn")
                        ps = pssc.tile([128, 2, SEG], F32, tag="sc",
                                       name=f"sc_{c}_{t}_{jj}")
                        # both K=64 row-group score matmuls issued adjacently
                        # so they run concurrently; the full-array causal
                        # mask matmuls (+= -30 where j > i, via cm.T @ I)
                        # trail them and close the accumulation groups.
                        for par in range(2):
                            lo, hi = par * 64, par * 64 + 64
                            nc.tensor.matmul(
                                ps[:, par, col0:],
                                rot[(c, jj // 4)][lo:hi, 1,
                                                  (jj % 4) * 128:(jj % 4) * 128 + 128],
                                rot[(c, t)][lo:hi, 0, col0:],
                                start=True, stop=(r < 0))
                        if r >= 0:
                            for par in range(2):
                                nc.tensor.matmul(
                                    ps[:, par, col0:col0 + 128],
                                    cm[:], i128[:], start=False, stop=True)
                        nc.scalar.activation(
                            a[:, :, col0:], ps[:, :, col0:],
                            mybir.ActivationFunctionType.Exp)
                        for _ in range(per_block):
                            next(filler, None)
                        for par in range(2):
                            nc.tensor.matmul(
                                pav[0:65, par, col0:],
                                v2[jj // 2][:, jj % 2, 2 * c + par, 0:65],
                                a[:, par, col0:],
                                start=(jj == 0), stop=(jj == njt - 1))
                    ot = persist.tile([128, SEG], BF16, tag=f"outt_{c}_{t}")
                    outt[(c, t)] = ot
                    # single copy out of PSUM frees the accumulator banks for
                    # the next (c,t) j-loop; row 64 of each par is the denom
                    last = (c == 1 and t == NSEG - 1)
                    # den must sit at base partition 0 for the custom DVE
                    # recip (HW reads garbage on shifted-base custom ops)
                    rec = work.tile([1, 2, SEG], F32, tag="rec")
                    if last:
                        # tail: skip the u staging copy (nothing is waiting
                        # on the pav banks); normalize straight from PSUM
                        u = pav
                        nc.scalar.copy(rec[:], pav[64:65, :, :])
                    else:
                        # u-copy frees the AV accumulator banks for the
                        # next chunk's j-loop
                        u = work.tile([65, 2, SEG], F32, tag="uav")
                        nc.any.tensor_copy(u[:], pav[0:65, :, :])
                        nc.any.tensor_copy(rec[:], u[64:65, :, :])
                    nc.vector.reciprocal_approx_fast(
                        rec.rearrange("p a b -> p (a b)"),
                        rec.rearrange("p a b -> p (a b)"))
                    bc = work.tile([64, 2, SEG], F32, tag="bc")
                    nc.gpsimd.partition_broadcast(
                        bc.rearrange("p a b -> p (a b)"),
                        rec.rearrange("p a b -> p (a b)"))
                    for par in range(2):
                        nc.vector.tensor_tensor(
                            ot[par * 64:par * 64 + 64, :],
                            u[0:64, par, :], bc[:, par, :],
                            mybir.AluOpType.mult)
                    if not last:
                        # dependency-free trickle matmuls scheduled at the
                        # chunk boundary: they bridge the pssc/pav handoff
                        # stall so the HAM clock-gate never re-throttles
                        # (a ~1us PE idle here costs ~3.4us of half-clock
                        # matmuls right after)
                        wtk = psmm.tile([128, SEG], F32, tag="mm",
                                        name=f"wtk_{c}_{t}")
                        for w in range(6):
                            nc.tensor.matmul(wtk[:64, 0:64], wmt[:, 0:64],
                                             wmt[:, 0:64],
                                             start=(w == 0), stop=(w == 5))
                for _ in filler:
                    pass
                if t == NSEG - 1:
                    # keep the PE (and its HAM clock) warm through the
                    # final normalize chain, then emit only the co=1 half
                    wps2 = psmm.tile([128, SEG], F32, tag="mm", name="warm2")
                    for w in range(24):
                        nc.tensor.matmul(wps2[:, 0:128], wmt[:], wmt[:],
                                         start=(w == 0), stop=(w == 23))
                    _emit_y(t, cos=(1,))

    nc.compile()
    return nc


def _get_program(with_qk_bias=False):
    if with_qk_bias not in _PROGRAMS:
        _PROGRAMS[with_qk_bias] = _build_program(with_qk_bias)
    return _PROGRAMS[with_qk_bias]


def _host_prep(x, wq, bq, wk, bk, wv, bv, wo, bo):
    """Build the 8 per-core input maps (all host-side numpy, cheap)."""
    bf = ml_dtypes.bfloat16
    x = np.asarray(x, np.float32)
    wq = np.asarray(wq, np.float32)
    wk = np.asarray(wk, np.float32)
    wv = np.asarray(wv, np.float32)
    wo = np.asarray(wo, np.float32)
    bq = np.asarray(bq, np.float32)
    bk = np.asarray(bk, np.float32)

    # rope tables, permuted-layout: partition p -> pair index m = p % 32,
    # first half of each 64-block (p%64<32) holds "evens", second "odds".
    # The 1/sqrt(64) softmax scale is folded into wq/bq, so q and k share
    # these tables; they are duplicated along a middle dim of 2 so one
    # vector op handles the q and k tiles together.
    m = np.arange(32, dtype=np.float64)
    inv_freq = 1.0 / (10000.0 ** (2.0 * m / HD))  # [32]
    pos = np.arange(S, dtype=np.float64)
    ang = pos[None, :] * inv_freq[:, None]  # [32, S]
    cos32 = np.cos(ang)
    sin32 = np.sin(ang)
    p = np.arange(128)
    cfull = cos32[p % 32, :]  # [128, S]
    sgn = np.where((p % 64) < 32, -1.0, 1.0)[:, None]
    sfull = sin32[p % 32, :] * sgn
    cos2 = np.ascontiguousarray(
        np.broadcast_to(cfull[:, None, :], (128, 2, S))).astype(bf)
    sin2 = np.ascontiguousarray(
        np.broadcast_to(sfull[:, None, :], (128, 2, S))).astype(bf)

    # scores[j, i'] += cmask[i', j] (cmask.T @ I128): -30 where j > i'
    cmask = np.where(np.arange(128)[None, :] > np.arange(128)[:, None],
                     -30.0, 0.0).astype(bf)
    i128 = np.eye(128, dtype=np.float32).astype(bf)

    scale = 1.0 / np.sqrt(HD)
    in_maps = []
    for core in range(NCORES):
        b, g = divmod(core, HEADS_PER_CORE)
        # permuted columns for q/k: per head [evens, odds]
        colmap = np.concatenate([
            (4 * g + hl) * HD + np.concatenate([np.arange(0, HD, 2),
                                                np.arange(1, HD, 2)])
            for hl in range(HEADS_PER_CORE)
        ])  # [256] global col indices
        vcols = np.arange(g * DP, (g + 1) * DP)

        xt = np.ascontiguousarray(
            x[b].T.reshape(KO, 128, S).transpose(1, 0, 2)).astype(bf)
        wq_t = np.ascontiguousarray(
            (wq[:, colmap] * scale).reshape(KO, 128, DP)
            .transpose(1, 0, 2)).astype(bf)
        wk_t = np.ascontiguousarray(
            wk[:, colmap].reshape(KO, 128, DP).transpose(1, 0, 2)).astype(bf)
        wv_t = np.ascontiguousarray(
            wv[:, vcols].reshape(KO, 128, DP).transpose(1, 0, 2)).astype(bf)
        wo_t = np.ascontiguousarray(
            wo[vcols, :].reshape(2, 128, D).transpose(1, 0, 2)).astype(bf)
        bq_t = np.ascontiguousarray(
            (bq[colmap] * scale).reshape(2, 128).T).astype(np.float32)
        bk_t = np.ascontiguousarray(bk[colmap].reshape(2, 128).T).astype(np.float32)

        in_maps.append({
            "xt": xt, "wqt": wq_t, "wkt": wk_t, "wvt": wv_t, "wot": wo_t,
            "bqt": bq_t, "bkt": bk_t,
            "cos2": cos2, "sin2": sin2, "cmask": cmask, "i128": i128,
        })
    return in_maps


def _run(nc, in_maps):
    if os.environ.get("BASS_SIM"):
        from concourse.bass_interp import CoreSim
        outs = []
        ncores = int(os.environ.get("BASS_SIM_CORES", "8"))
        for i, m in enumerate(in_maps[:ncores]):
            sim = CoreSim(nc, require_finite=False, require_nnan=False)
            for k, v in m.items():
                sim.tensor(k)[:] = v
            sim.simulate(check_with_hw=False)
            outs.append({"y": np.array(sim.tensor("y")),
                         "y2": np.array(sim.tensor("y2"))})
        while len(outs) < len(in_maps):
            outs.append({"y": np.zeros((S, D), ml_dtypes.bfloat16),
                         "y2": np.zeros((SEG, D), ml_dtypes.bfloat16)})
        return outs
    from concourse.bass_utils import run_bass_kernel_spmd
    res = run_bass_kernel_spmd(nc, in_maps, list(range(NCORES)))
    return res.results


def kernel(x, wq, bq, wk, bk, wv, bv, wo, bo):
    with_qk_bias = bool(np.any(np.asarray(bq)) or np.any(np.asarray(bk)))
    nc = _get_program(with_qk_bias)
    in_maps = _host_prep(x, wq, bq, wk, bk, wv, bv, wo, bo)
    results = _run(nc, in_maps)
    bv = np.asarray(bv, np.float32)
    bo = np.asarray(bo, np.float32)
    wo_f = np.asarray(wo, np.float32)
    corr = bv @ wo_f + bo  # [D]
    y = np.zeros((B, S, D), np.float32)
    for core in range(NCORES):
        b = core // HEADS_PER_CORE
        y[b] += np.asarray(results[core]["y"], np.float32)
        y[b, S - SEG:] += np.asarray(results[core]["y2"], np.float32)
    y += corr[None, None, :]
    return y


# revision 40
# speedup vs baseline: 1.0083x; 1.0083x over previous
"""Multi-head attention (B=2, S=2048, D=1024, H=16, RoPE, causal) on 8 trn2 cores.

Sharding: core = b*4 + g  ->  batch b in {0,1}, head-group g (4 heads of 64 dims).
Each core computes q/k/v projections for its 4 heads, RoPE, causal attention,
and a partial output projection (its 256 rows of wo). Host sums the 4 partials
per batch and adds the analytic bias correction bv@wo + bo (softmax rows sum
to 1, so bv contributes a constant vector; bo is a constant vector).

Device layouts are feature-on-partition ("transposed"):
  xt [128, 8, 2048]   xt[p, ko, s] = x[b, s, ko*128 + p]            (bf16)
  qT/kT computed directly as [d', s]; RoPE pair-swap becomes a 32-block
  partition swap because wq/wk columns are host-permuted to [evens|odds]
  per head. The swap runs as 4 SBUF->SBUF partition-block DMAs (p ^ 32),
  freeing the PE of permutation matmuls. The softmax 1/sqrt(64) scale is
  folded into wq/bq on the host so q and k share one cos/sin table pair
  (duplicated along a middle dim so one DVE op covers q and k together).
  scoresT[j, i] = kT.T @ qT per head; two heads (one 128-part chunk) run as
  concurrent K=64 row-group matmuls into the two banks of one [128,1024]
  PSUM tile, so exp / mask / normalize handle both heads per op.
  Softmax skips max-subtraction (|score| <~ 8 here); exp on ACT. The
  denominator comes free from a ones-column appended to v in the AV matmul
  (out rows 0..63 = v.T @ attnT, row 64 = column sums). The reciprocal runs
  on the DVE (reciprocal_approx_fast) so the ACT queue carries nothing but
  the block exps; the normalize multiplies run on the otherwise-idle GpSimd.
  Causality: blocks above the diagonal are skipped, diagonal blocks compute
  only columns >= 128*r and mask a single 128-wide strip via a -30 matmul.
  y[s, e] = outT.T @ wo accumulated over the 2 c-chunks, DMA'd PSUM->DRAM.
"""

import os

import numpy as np
import ml_dtypes

import concourse.bass as bass
import concourse.bacc as bacc
import concourse.tile as tile
from concourse import library_config, mybir

B = 2
S = 2048
D = 1024
H = 16
HD = 64
NCORES = 8
HEADS_PER_CORE = 4
DP = 256  # head dims per core
SEG = 512  # i-seg / s-seg size
NSEG = S // SEG  # 4
NST = S // 128  # 16 s-tiles / j-tiles
KO = D // 128  # 8 contraction tiles

F32 = mybir.dt.float32
BF16 = mybir.dt.bfloat16

_PROGRAMS = {}


def _build_program(with_qk_bias):
    nc = bacc.Bacc("TRN2", target_bir_lowering=False, debug=False)

    xt_d = nc.dram_tensor("xt", [128, KO, S], BF16, kind="ExternalInput")
    wq_d = nc.dram_tensor("wqt", [128, KO, DP], BF16, kind="ExternalInput")
    wk_d = nc.dram_tensor("wkt", [128, KO, DP], BF16, kind="ExternalInput")
    wv_d = nc.dram_tensor("wvt", [128, KO, DP], BF16, kind="ExternalInput")
    wo_d = nc.dram_tensor("wot", [128, 2, D], BF16, kind="ExternalInput")
    bq_d = nc.dram_tensor("bqt", [128, 2], F32, kind="ExternalInput")
    bk_d = nc.dram_tensor("bkt", [128, 2], F32, kind="ExternalInput")
    cos_d = nc.dram_tensor("cos2", [128, 2, S], BF16, kind="ExternalInput")
    sin_d = nc.dram_tensor("sin2", [128, 2, S], BF16, kind="ExternalInput")
    cm_d = nc.dram_tensor("cmask", [128, 128], BF16, kind="ExternalInput")
    i128_d = nc.dram_tensor("i128", [128, 128], BF16, kind="ExternalInput")
    y_d = nc.dram_tensor("y", [S, D], BF16, kind="ExternalOutput")
    # co=0 partial of the last segment's y, emitted during the last chunk's
    # attention so the post-norm tail only runs the co=1 half (host adds it)
    y2_d = nc.dram_tensor("y2", [SEG, D], BF16, kind="ExternalOutput")

    with tile.TileContext(nc) as tc:
        with (
            tc.tile_pool(name="const", bufs=1) as const,
            tc.tile_pool(name="persist", bufs=1) as persist,
            tc.tile_pool(name="work", bufs=4) as work,
            tc.tile_pool(name="psmm", bufs=2, space="PSUM") as psmm,
            tc.tile_pool(name="pssc", bufs=2, space="PSUM") as pssc,
            tc.tile_pool(name="psacc", bufs=1, space="PSUM") as psacc,
        ):
            # one gpsimd library serving BOTH partition_broadcast and
            # tensor_tensor: a mid-kernel LIBRARY_RELOAD costs ~6us of
            # hidden ucode DMA, so the normalize chain must never thrash
            nc.gpsimd.load_library(library_config.proxy)

            # ---- constants ----  (DMA order = first-needed first)
            wq = const.tile([128, KO, DP], BF16, tag="wq")
            nc.sync.dma_start(wq[:], wq_d[:])
            xt = []
            xt0h = []  # first seg split in two ko-halves for an early start
            for h in range(2):
                xh = const.tile([128, KO // 2, SEG], BF16, tag=f"xt0h{h}",
                                name=f"xt0h{h}")
                nc.sync.dma_start(
                    xh[:], xt_d[:, h * (KO // 2):(h + 1) * (KO // 2), 0:SEG])
                xt0h.append(xh)
            for t in range(NSEG):
                if t == 0:
                    xt.append(None)
                    continue
                xt.append(const.tile([128, KO, SEG], BF16, tag=f"xt{t}",
                                     name=f"xt{t}"))
            wk = const.tile([128, KO, DP], BF16, tag="wk")
            nc.sync.dma_start(wk[:], wk_d[:])

            def xt_ap(t, ko):
                if t == 0:
                    return xt0h[ko // (KO // 2)][:, ko % (KO // 2), :]
                return xt[t][:, ko, :]
            if with_qk_bias:
                bq = const.tile([128, 2], F32, tag="bq")
                nc.sync.dma_start(bq[:], bq_d[:])
                bk = const.tile([128, 2], F32, tag="bk")
                nc.sync.dma_start(bk[:], bk_d[:])
            ctab = const.tile([128, 2, S], BF16, tag="ctab")
            nc.sync.dma_start(ctab[:], cos_d[:])
            stab = const.tile([128, 2, S], BF16, tag="stab")
            nc.sync.dma_start(stab[:], sin_d[:])
            wv = const.tile([128, KO, DP], BF16, tag="wv")
            nc.sync.dma_start(wv[:], wv_d[:])
            cm = const.tile([128, 128], BF16, tag="cm")
            nc.sync.dma_start(cm[:], cm_d[:])
            i128 = const.tile([128, 128], BF16, tag="i128")
            nc.sync.dma_start(i128[:], i128_d[:])
            for t in range(1, NSEG):
                nc.sync.dma_start(xt[t][:], xt_d[:, :, t * SEG:(t + 1) * SEG])
            wo = const.tile([128, 2, D], BF16, tag="wo")
            nc.sync.dma_start(wo[:], wo_d[:])

            # ---- PE warmup: ~4.5us of dummy matmuls while DMAs stream,
            # so the HAM clock-gate is at 8/8 when real work starts ----
            wmt = work.tile([128, 128], BF16, tag="wmt")
            nc.vector.memset(wmt[:], 0.0)
            wps = psmm.tile([128, SEG], F32, tag="mm", name="warm")
            for w in range(48):
                nc.tensor.matmul(wps[:, 0:128], wmt[:], wmt[:],
                                 start=(w == 0), stop=(w == 47))

            # ---- per-segment pipeline: projections -> attention -> y ----
            rot = {}        # (c, t) -> [128, 2(q/k), SEG] bf16
            v2 = [None] * (NST // 2)  # half-seg -> [128, 2, 4, 66] bf16
            outt = {}

            def _emit_y(yt, cos=(0, 1), ydst=None, yrow0=None):
                if yt < 0:
                    return
                for sl in range(4):
                    st = 4 * yt + sl
                    for es in range(2):
                        py = psmm.tile([128, SEG], F32, tag="mm",
                                       name=f"py_{st}_{es}_{cos[0]}")
                        for i, co in enumerate(cos):
                            nc.tensor.matmul(
                                py[:],
                                outt[(co, yt)][:, sl * 128:sl * 128 + 128],
                                wo[:, co, es * SEG:(es + 1) * SEG],
                                start=(i == 0), stop=(i == len(cos) - 1))
                        ysb = work.tile([128, SEG], BF16, tag="ysb")
                        nc.any.tensor_copy(ysb[:], py[:])
                        dst = y_d if ydst is None else ydst
                        r0 = st * 128 if yrow0 is None else yrow0 + sl * 128
                        nc.sync.dma_start(
                            dst[r0:r0 + 128, es * SEG:(es + 1) * SEG],
                            ysb[:])

            def proj_steps(t):
                """Generator: q/k/v projections + rope for segment t,
                yielded in small PE-group steps so the caller can weave
                them between attention blocks (filler for the exp-gated
                AV waits)."""
                for c in range(2):
                    qk_sb = work.tile([128, 2, SEG], BF16, tag="qk_sb")
                    for qk, (w_sb, b_nm) in enumerate(
                            ((wq, "bq"), (wk, "bk"))):
                        pp = psmm.tile([128, SEG], F32, tag="mm",
                                       name=f"pp{qk}_{c}_{t}")
                        for ko in range(KO):
                            nc.tensor.matmul(
                                pp[:],
                                w_sb[:, ko, c * 128:(c + 1) * 128],
                                xt_ap(t, ko),
                                start=(ko == 0),
                                stop=(ko == KO - 1),
                            )
                            if ko % 3 == 2:
                                yield
                        if with_qk_bias:
                            b_sb = bq if b_nm == "bq" else bk
                            nc.vector.tensor_scalar_add(
                                qk_sb[:, qk, :], pp[:], b_sb[:, c:c + 1])
                        else:
                            nc.any.tensor_copy(qk_sb[:, qk, :], pp[:])
                        yield
                    # pair-swap (p ^ 32) via 4 SBUF->SBUF partition DMAs
                    swp = work.tile([128, 2, SEG], BF16, tag="swp")
                    for g in range(4):
                        src = g ^ 1
                        nc.sync.dma_start(
                            swp[g * 32:(g + 1) * 32, :, :],
                            qk_sb[src * 32:(src + 1) * 32, :, :])
                    t1 = work.tile([128, 2, SEG], BF16, tag="t1")
                    nc.vector.tensor_tensor(
                        t1[:], qk_sb[:], ctab[:, :, t * SEG:(t + 1) * SEG],
                        mybir.AluOpType.mult)
                    t2 = work.tile([128, 2, SEG], BF16, tag="t2")
                    nc.vector.tensor_tensor(
                        t2[:], swp[:], stab[:, :, t * SEG:(t + 1) * SEG],
                        mybir.AluOpType.mult)
                    rt = persist.tile([128, 2, SEG], BF16, tag=f"rot_{c}_{t}")
                    nc.vector.tensor_tensor(
                        rt[:], t1[:], t2[:], mybir.AluOpType.add)
                    rot[(c, t)] = rt
                    yield
                # v projection for the 4 s-tiles of this seg, 2 per bank
                for half in range(2):
                    hs = 2 * t + half
                    pv = psmm.tile([128, SEG], F32, tag="mm", name=f"pv_{hs}")
                    for sl in range(2):
                        st = 4 * t + 2 * half + sl
                        for ko in range(KO):
                            nc.tensor.matmul(
                                pv[:, sl * DP:sl * DP + DP],
                                xt_ap(t, ko)[:, (st % NSEG) * 128:
                                             (st % NSEG) * 128 + 128],
                                wv[:, ko, :],
                                start=(ko == 0),
                                stop=(ko == KO - 1),
                            )
                            if ko % 3 == 2:
                                yield
                    v_t = persist.tile([128, 2, HEADS_PER_CORE, 66], BF16,
                                       tag=f"v2_{hs}")
                    nc.vector.memset(v_t[:, :, :, 64:66], 1.0)
                    nc.any.tensor_copy(
                        v_t[:, :, :, 0:64],
                        pv[:].rearrange("p (s h d) -> p s h d",
                                        s=2, h=HEADS_PER_CORE))
                    v2[hs] = v_t
                    yield

            for t in range(NSEG):
                if t == 0:
                    for _ in proj_steps(0):
                        pass
                # next segment's projections are woven into this segment's
                # attention as PE filler (consumed between scores and AV)
                filler = proj_steps(t + 1) if t + 1 < NSEG else iter(())
                nsteps = 30  # upper bound on proj_steps yields
                nblocks = 2 * (4 * t + 4)
                per_block = max(1, -(-nsteps // nblocks))
                # attention for this seg (both chunks); y(t-1) emitted
                # between the chunks to fill PE bubbles at the c=0 tail
                for c in range(2):
                    if c == 1:
                        _emit_y(t - 1)
                        if t == NSEG - 1:
                            # co=0 half of the last segment's y runs as
                            # filler during this chunk's attention
                            _emit_y(t, cos=(0,), ydst=y2_d, yrow0=0)
                    pav = psacc.tile([128, 2, SEG], F32, tag="av",
                                     name=f"av_{c}_{t}")
                    njt = 4 * t + 4
                    for jj in range(njt):
                        r = jj - 4 * t  # >= 0 on diagonal blocks
                        col0 = max(0, r) * 128  # first useful i-column
                        a = work.tile([128, 2, SEG], BF16, tag="attn")
                        ps = pssc.tile([128, 2, SEG], F32, tag="sc",
                                       name=f"sc_{c}_{t}_{jj}")
                        # both K=64 row-group score matmuls issued adjacently
                        # so they run concurrently; the full-array causal
                        # mask matmuls (+= -30 where j > i, via cm.T @ I)
                        # trail them and close the accumulation groups.
                        for par in range(2):
                            lo, hi = par * 64, par * 64 + 64
                            nc.tensor.matmul(
                                ps[:, par, col0:],
                                rot[(c, jj // 4)][lo:hi, 1,
                                                  (jj % 4) * 128:(jj % 4) * 128 + 128],
                                rot[(c, t)][lo:hi, 0, col0:],
                                start=True, stop=(r < 0))
                        if r >= 0:
                            for par in range(2):
                                nc.tensor.matmul(
                                    ps[:, par, col0:col0 + 128],
                                    cm[:], i128[:], start=False, stop=True)
                        nc.scalar.activation(
                            a[:, :, col0:], ps[:, :, col0:],
                            mybir.ActivationFunctionType.Exp)
                        for _ in range(per_block):
                            next(filler, None)
                        for par in range(2):
                            nc.tensor.matmul(
                                pav[0:65, par, col0:],
                                v2[jj // 2][:, jj % 2, 2 * c + par, 0:65],
                                a[:, par, col0:],
                                start=(jj == 0), stop=(jj == njt - 1))
                    ot = persist.tile([128, SEG], BF16, tag=f"outt_{c}_{t}")
                    outt[(c, t)] = ot
                    # single copy out of PSUM frees the accumulator banks for
                    # the next (c,t) j-loop; row 64 of each par is the denom
                    last = (c == 1 and t == NSEG - 1)
                    # den must sit at base partition 0 for the custom DVE
                    # recip (HW reads garbage on shifted-base custom ops)
                    rec = work.tile([1, 2, SEG], F32, tag="rec")
                    if last:
                        # tail: skip the u staging copy (nothing is waiting
                        # on the pav banks); normalize straight from PSUM
                        u = pav
                        nc.scalar.copy(rec[:], pav[64:65, :, :])
                    else:
                        # u-copy frees the AV accumulator banks for the
                        # next chunk's j-loop; the den copy reads PSUM
                        # directly so it runs in parallel, not chained
                        u = work.tile([65, 2, SEG], F32, tag="uav")
                        nc.any.tensor_copy(u[:], pav[0:65, :, :])
                        nc.any.tensor_copy(rec[:], pav[64:65, :, :])
                    nc.vector.reciprocal_approx_fast(
                        rec.rearrange("p a b -> p (a b)"),
                        rec.rearrange("p a b -> p (a b)"))
                    bc = work.tile([64, 2, SEG], F32, tag="bc")
                    nc.gpsimd.partition_broadcast(
                        bc.rearrange("p a b -> p (a b)"),
                        rec.rearrange("p a b -> p (a b)"))
                    for par in range(2):
                        nc.vector.tensor_tensor(
                            ot[par * 64:par * 64 + 64, :],
                            u[0:64, par, :], bc[:, par, :],
                            mybir.AluOpType.mult)
                    if not last:
                        # dependency-free trickle matmuls scheduled at the
                        # chunk boundary: they bridge the pssc/pav handoff
                        # stall so the HAM clock-gate never re-throttles
                        # (a ~1us PE idle here costs ~3.4us of half-clock
                        # matmuls right after)
                        wtk = psmm.tile([128, SEG], F32, tag="mm",
                                        name=f"wtk_{c}_{t}")
                        for w in range(6):
                            nc.tensor.matmul(wtk[:64, 0:64], wmt[:, 0:64],
                                             wmt[:, 0:64],
                                             start=(w == 0), stop=(w == 5))
                for _ in filler:
                    pass
                if t == NSEG - 1:
                    # keep the PE (and its HAM clock) warm through the
                    # final normalize chain, then emit only the co=1 half
                    wps2 = psmm.tile([128, SEG], F32, tag="mm", name="warm2")
                    for w in range(24):
                        nc.tensor.matmul(wps2[:, 0:128], wmt[:], wmt[:],
                                         start=(w == 0), stop=(w == 23))
                    _emit_y(t, cos=(1,))

    nc.compile()
    return nc


def _get_program(with_qk_bias=False):
    if with_qk_bias not in _PROGRAMS:
        _PROGRAMS[with_qk_bias] = _build_program(with_qk_bias)
    return _PROGRAMS[with_qk_bias]


def _host_prep(x, wq, bq, wk, bk, wv, bv, wo, bo):
    """Build the 8 per-core input maps (all host-side numpy, cheap)."""
    bf = ml_dtypes.bfloat16
    x = np.asarray(x, np.float32)
    wq = np.asarray(wq, np.float32)
    wk = np.asarray(wk, np.float32)
    wv = np.asarray(wv, np.float32)
    wo = np.asarray(wo, np.float32)
    bq = np.asarray(bq, np.float32)
    bk = np.asarray(bk, np.float32)

    # rope tables, permuted-layout: partition p -> pair index m = p % 32,
    # first half of each 64-block (p%64<32) holds "evens", second "odds".
    # The 1/sqrt(64) softmax scale is folded into wq/bq, so q and k share
    # these tables; they are duplicated along a middle dim of 2 so one
    # vector op handles the q and k tiles together.
    m = np.arange(32, dtype=np.float64)
    inv_freq = 1.0 / (10000.0 ** (2.0 * m / HD))  # [32]
    pos = np.arange(S, dtype=np.float64)
    ang = pos[None, :] * inv_freq[:, None]  # [32, S]
    cos32 = np.cos(ang)
    sin32 = np.sin(ang)
    p = np.arange(128)
    cfull = cos32[p % 32, :]  # [128, S]
    sgn = np.where((p % 64) < 32, -1.0, 1.0)[:, None]
    sfull = sin32[p % 32, :] * sgn
    cos2 = np.ascontiguousarray(
        np.broadcast_to(cfull[:, None, :], (128, 2, S))).astype(bf)
    sin2 = np.ascontiguousarray(
        np.broadcast_to(sfull[:, None, :], (128, 2, S))).astype(bf)

    # scores[j, i'] += cmask[i', j] (cmask.T @ I128): -30 where j > i'
    cmask = np.where(np.arange(128)[None, :] > np.arange(128)[:, None],
                     -30.0, 0.0).astype(bf)
    i128 = np.eye(128, dtype=np.float32).astype(bf)

    scale = 1.0 / np.sqrt(HD)
    in_maps = []
    for core in range(NCORES):
        b, g = divmod(core, HEADS_PER_CORE)
        # permuted columns for q/k: per head [evens, odds]
        colmap = np.concatenate([
            (4 * g + hl) * HD + np.concatenate([np.arange(0, HD, 2),
                                                np.arange(1, HD, 2)])
            for hl in range(HEADS_PER_CORE)
        ])  # [256] global col indices
        vcols = np.arange(g * DP, (g + 1) * DP)

        xt = np.ascontiguousarray(
            x[b].T.reshape(KO, 128, S).transpose(1, 0, 2)).astype(bf)
        wq_t = np.ascontiguousarray(
            (wq[:, colmap] * scale).reshape(KO, 128, DP)
            .transpose(1, 0, 2)).astype(bf)
        wk_t = np.ascontiguousarray(
            wk[:, colmap].reshape(KO, 128, DP).transpose(1, 0, 2)).astype(bf)
        wv_t = np.ascontiguousarray(
            wv[:, vcols].reshape(KO, 128, DP).transpose(1, 0, 2)).astype(bf)
        wo_t = np.ascontiguousarray(
            wo[vcols, :].reshape(2, 128, D).transpose(1, 0, 2)).astype(bf)
        bq_t = np.ascontiguousarray(
            (bq[colmap] * scale).reshape(2, 128).T).astype(np.float32)
        bk_t = np.ascontiguousarray(bk[colmap].reshape(2, 128).T).astype(np.float32)

        in_maps.append({
            "xt": xt, "wqt": wq_t, "wkt": wk_t, "wvt": wv_t, "wot": wo_t,
            "bqt": bq_t, "bkt": bk_t,
            "cos2": cos2, "sin2": sin2, "cmask": cmask, "i128": i128,
        })
    return in_maps


def _run(nc, in_maps):
    if os.environ.get("BASS_SIM"):
        from concourse.bass_interp import CoreSim
        outs = []
        ncores = int(os.environ.get("BASS_SIM_CORES", "8"))
        for i, m in enumerate(in_maps[:ncores]):
            sim = CoreSim(nc, require_finite=False, require_nnan=False)
            for k, v in m.items():
                sim.tensor(k)[:] = v
            sim.simulate(check_with_hw=False)
            outs.append({"y": np.array(sim.tensor("y")),
                         "y2": np.array(sim.tensor("y2"))})
        while len(outs) < len(in_maps):
            outs.append({"y": np.zeros((S, D), ml_dtypes.bfloat16),
                         "y2": np.zeros((SEG, D), ml_dtypes.bfloat16)})
        return outs
    from concourse.bass_utils import run_bass_kernel_spmd
    res = run_bass_kernel_spmd(nc, in_maps, list(range(NCORES)))
    return res.results


def kernel(x, wq, bq, wk, bk, wv, bv, wo, bo):
    with_qk_bias = bool(np.any(np.asarray(bq)) or np.any(np.asarray(bk)))
    nc = _get_program(with_qk_bias)
    in_maps = _host_prep(x, wq, bq, wk, bk, wv, bv, wo, bo)
    results = _run(nc, in_maps)
    bv = np.asarray(bv, np.float32)
    bo = np.asarray(bo, np.float32)
    wo_f = np.asarray(wo, np.float32)
    corr = bv @ wo_f + bo  # [D]
    y = np.zeros((B, S, D), np.float32)
    for core in range(NCORES):
        b = core // HEADS_PER_CORE
        y[b] += np.asarray(results[core]["y"], np.float32)
        y[b, S - SEG:] += np.asarray(results[core]["y2"], np.float32)
    y += corr[None, None, :]
    return y
